# revision 10
# baseline (speedup 1.0000x reference)
"""DePatchEmbed (patch scatter) for 8 trn2 NeuronCores.

Math (N=4, C=256, H=W=256, p=8, Gi=Gj=32, num_patches=1024, dim=16384):
    out[n, c, i*8+a, j*8+b] = x[n, j*32+i, c*64 + a*8 + b]

Sharding (scheme C): core = ih in [0,8) owns i in [ih*4, ih*4+4) — all
batches, all channels.
  per-core input : x.reshape(4,32,32,16384)[:, :, ih*4:(ih+1)*4, :]
                   -> x_core (n=4, j=32, i_loc=4, dim=16384)   32 MB
  per-core output: out[:, :, ih*32:(ih+1)*32, :] -> (4, 256, 32, 256)

SBUF partition p = n*32 + cb owns batch n, channel block cb (8 channels),
so DRAM load chunks are 512 contiguous elements (2 KB descriptors) and the
permutation stays within-partition.  Store descriptors are 8 KB.

Pipeline per i_loc (4 steps):
  loads  (HWDGE/SP): per j: [n:4][cb:32][e:512 contig] -> t_in[:, j*512:]
                     2 KB descs, 256 KB per DMA, 32 DMAs -> 2 tiles (j half)
  shuffle (DVE):     per (c2, jh): (q, j, a, b) in strides (512,8,1) ->
                     out strides (8,256,1)
  stores (HWDGE/ACT): per c2: [n:4][cb:32][2048 contig]  8 KB descs, 1 MB
"""

import numpy as np

import concourse.bacc as bacc
import concourse.bass as bass  # noqa: F401
import concourse.mybir as mybir
import concourse.tile as tile
from concourse.bass_utils import run_bass_kernel_spmd

N, C, H, W = 4, 256, 256, 256
P = 8          # patch size
GI = 32        # row blocks (i, scanned fastest in k)
GJ = 32        # col blocks (j)
IL = 4         # i-values per core (GI / 8 cores)
JH = GJ // 2   # j per in-tile

_NC = None


def _build():
    # Bacc (not plain Bass): its finalize() runs generate_event_semaphores,
    # which splits multi-wait instructions (HW allows 1 sync-wait per DMA).
    nc = bacc.Bacc()
    x = nc.dram_tensor(
        "x", [N, GJ, IL, 16384], mybir.dt.float32, kind="ExternalInput"
    )
    o = nc.dram_tensor("o", [N, C, GI, W], mybir.dt.float32, kind="ExternalOutput")
    # x5[i_loc, j, n, cb, e]: per (i_loc, j) a 3-dim AP [n][cb][e contig]
    x5 = x.rearrange("n j i (cb e) -> i j n cb e", cb=GI)
    # o5[i_loc, c2, n, cb, rw]: per (i_loc, c2) a 3-dim AP [n][cb][2048 contig]
    o5 = o.rearrange("n (cb c2) (i r) w -> i c2 n cb (r w)", c2=P, r=P)

    with tile.TileContext(nc) as tc:
        with (
            tc.tile_pool(name="tin", bufs=4) as pin,
            tc.tile_pool(name="tout", bufs=6) as pout,
        ):
            for i in range(IL):
                t_ins = []
                for jh in range(2):
                    t_in = pin.tile([128, JH * 512], mybir.dt.float32)
                    for jl in range(JH):
                        j = jh * JH + jl
                        nc.sync.dma_start(
                            out=t_in[:, jl * 512 : (jl + 1) * 512], in_=x5[i, j]
                        )
                    t_ins.append(t_in)
                # t_in laid out (j_loc, c2, a, b); views (q, c2, j_loc, a, b)
                tin_v = [
                    t[:].rearrange("q (j c2 a b) -> q c2 j a b", c2=P, a=P, b=P)
                    for t in t_ins
                ]
                for c2 in range(P):
                    t_out = pout.tile([128, 2048], mybir.dt.float32)
                    # t_out laid out (a, j, b); view (q, j, a, b) per j half
                    tout_v = t_out[:].rearrange(
                        "q (a jh j b) -> q jh j a b", a=P, jh=2, j=JH, b=P
                    )
                    for jh in range(2):
                        nc.vector.tensor_copy(
                            out=tout_v[:, jh], in_=tin_v[jh][:, c2]
                        )
                    nc.scalar.dma_start(out=o5[i, c2], in_=t_out[:])
    nc.finalize()
    return nc


def _get_nc():
    global _NC
    if _NC is None:
        _NC = _build()
    return _NC


def _shard_inputs(x_np):
    v = x_np.reshape(N, GJ, GI, 16384)
    return [
        {"x": np.ascontiguousarray(v[:, :, ih * IL : (ih + 1) * IL, :])}
        for ih in range(8)
    ]


def _gather_outputs(results):
    out = np.empty((N, C, H, W), dtype=np.float32)
    for ih in range(8):
        out[:, :, ih * GI : (ih + 1) * GI, :] = results[ih]["o"]
    return out


def run(x_np, **spmd_kwargs):
    """Run on 8 cores; returns (out, BassKernelResults)."""
    nc = _get_nc()
    res = run_bass_kernel_spmd(
        nc, _shard_inputs(x_np), core_ids=list(range(8)), **spmd_kwargs
    )
    return _gather_outputs(res.results), res


def kernel(x, ori_shape=None, patch_size=None, **_):
    x_np = np.asarray(x, dtype=np.float32).reshape(N, 1024, 16384)
    out, _res = run(x_np)
    return out


# revision 13
# speedup vs baseline: 2.1807x; 2.1807x over previous
"""DePatchEmbed (patch scatter) for 8 trn2 NeuronCores.

Math (N=4, C=256, H=W=256, p=8, Gi=Gj=32, num_patches=1024, dim=16384):
    out[n, c, i*8+a, j*8+b] = x[n, j*32+i, c*64 + a*8 + b]

Sharding (scheme D): core = ih in [0,8) owns i in [ih*4, ih*4+4) — all
batches, all channels (pure row-block data parallelism, zero communication).
  per-core input : x.reshape(4,32,32,16384)[:, :, ih*4:(ih+1)*4, :]
                   packed on host as (j=32, i_loc=4, n=4, dim=16384)  32 MB
  per-core output: out[:, :, ih*32:(ih+1)*32, :] -> (4, 256, 32, 256) 32 MB

SBUF partition p = n*32 + cb owns (batch n, channel block cb of 8 channels).
With the (j, i, n, d) host packing, (n, cb) is a single uniform-stride DMA
dim (stride 512 elems), so loads are big (4 MB) with 2 KB descriptors and
the whole permutation stays within-partition.  Stores are 4 MB DMAs with
8 KB descriptors.

Per i_loc (4 steps):
  2 loads (HWDGE/SP):  per jh: [(n cb):512x128][j:16][e:512 contig]  4 MB
  16 copies (DVE):     per (c2l, jh): (q,j,a,b) strides (512,8,1)->(8,256,1)
  2 stores (HWDGE/ACT): per ch: [(n cb):65536x128][c2l:4][2048 contig] 4 MB
"""

import numpy as np

import concourse.bacc as bacc
import concourse.bass as bass  # noqa: F401
import concourse.mybir as mybir
import concourse.tile as tile
from concourse.bass_utils import run_bass_kernel_spmd

N, C, H, W = 4, 256, 256, 256
P = 8          # patch size
GI = 32        # row blocks (i, scanned fastest in k)
GJ = 32        # col blocks (j)
IL = 4         # i-values per core (GI / 8 cores)
JH = GJ // 2   # j per in-tile

_NC = None


def _build():
    # Bacc (not plain Bass): its finalize() runs generate_event_semaphores,
    # which splits multi-wait instructions (HW allows 1 sync-wait per DMA).
    nc = bacc.Bacc()
    x = nc.dram_tensor(
        "x", [GJ, IL, N, 16384], mybir.dt.float32, kind="ExternalInput"
    )
    o = nc.dram_tensor("o", [N, C, GI, W], mybir.dt.float32, kind="ExternalOutput")
    # x5[i_loc, jh, q=(n cb), j, e]: per (i_loc, jh) a 3-dim AP
    #   [[512,128],[262144,16],[1,512]]
    x5 = x.rearrange("(jh j) i n (cb e) -> i jh (n cb) j e", jh=2, cb=GI)
    # o5[i_loc, ch, q=(n cb), c2l, rw]: per (i_loc, ch) a 3-dim AP
    #   [[65536,128],[8192,4],[1,2048]]
    o5 = o.rearrange("n (cb ch c2l) (i r) w -> i ch (n cb) c2l (r w)", ch=2, c2l=4, r=P)

    with tile.TileContext(nc) as tc:
        with (
            tc.tile_pool(name="tin", bufs=3) as pin,
            tc.tile_pool(name="tout", bufs=2) as pout,
        ):
            for i in range(IL):
                t_ins = []
                for jh in range(2):
                    t_in = pin.tile([128, JH * 512], mybir.dt.float32)
                    nc.sync.dma_start(out=t_in[:], in_=x5[i, jh])
                    t_ins.append(t_in)
                # t_in laid out (j_loc, c2, a, b); view (q, c2, j_loc, a, b)
                tin_v = [
                    t[:].rearrange("q (j c2 a b) -> q c2 j a b", c2=P, a=P, b=P)
                    for t in t_ins
                ]
                for ch in range(2):
                    t_out = pout.tile([128, 4 * 2048], mybir.dt.float32)
                    # t_out laid out (c2l, a, j, b); view (q, c2l, jh, j, a, b)
                    tout_v = t_out[:].rearrange(
                        "q (c2l a jh j b) -> q c2l jh j a b",
                        c2l=4, a=P, jh=2, j=JH, b=P,
                    )
                    for c2l in range(4):
                        for jh in range(2):
                            nc.vector.tensor_copy(
                                out=tout_v[:, c2l, jh],
                                in_=tin_v[jh][:, ch * 4 + c2l],
                            )
                    nc.scalar.dma_start(out=o5[i, ch], in_=t_out[:])
    nc.finalize()
    return nc


def _get_nc():
    global _NC
    if _NC is None:
        _NC = _build()
    return _NC


def _shard_inputs(x_np):
    v = x_np.reshape(N, GJ, GI, 16384)
    return [
        {
            "x": np.ascontiguousarray(
                v[:, :, ih * IL : (ih + 1) * IL, :].transpose(1, 2, 0, 3)
            )
        }
        for ih in range(8)
    ]


def _gather_outputs(results):
    out = np.empty((N, C, H, W), dtype=np.float32)
    for ih in range(8):
        out[:, :, ih * GI : (ih + 1) * GI, :] = results[ih]["o"]
    return out


def run(x_np, **spmd_kwargs):
    """Run on 8 cores; returns (out, BassKernelResults)."""
    nc = _get_nc()
    res = run_bass_kernel_spmd(
        nc, _shard_inputs(x_np), core_ids=list(range(8)), **spmd_kwargs
    )
    return _gather_outputs(res.results), res


def kernel(x, ori_shape=None, patch_size=None, **_):
    x_np = np.asarray(x, dtype=np.float32).reshape(N, 1024, 16384)
    out, _res = run(x_np)
    return out


# revision 16
# speedup vs baseline: 2.3866x; 1.0944x over previous
"""DePatchEmbed (patch scatter) for 8 trn2 NeuronCores.

Math (N=4, C=256, H=W=256, p=8, Gi=Gj=32, num_patches=1024, dim=16384):
    out[n, c, i*8+a, j*8+b] = x[n, j*32+i, c*64 + a*8 + b]

Sharding (scheme D): core = ih in [0,8) owns i in [ih*4, ih*4+4) — all
batches, all channels (pure row-block data parallelism, zero communication).
  per-core input : x.reshape(4,32,32,16384)[:, :, ih*4:(ih+1)*4, :]
                   packed on host as (j=32, i_loc=4, n=4, dim=16384)  32 MB
  per-core output: out[:, :, ih*32:(ih+1)*32, :] -> (4, 256, 32, 256) 32 MB

SBUF partition p = n*32 + cb owns (batch n, channel block cb of 8 channels).
With the (j, i, n, d) host packing, (n, cb) is a single uniform-stride DMA
dim (stride 512 elems), so loads are big (4 MB) with 2 KB descriptors and
the whole permutation stays within-partition.  Stores are 4 MB DMAs with
8 KB descriptors.

Per i_loc (4 steps), 2 MB grain for short pipeline fill/drain:
  4 loads (HWDGE/SP):  per jq: [(n cb):512x128][j:8][e:512 contig]   2 MB
  32 copies (DVE):     per (c2, jq): (q,j,a,b) strides (512,8,1)->(8,256,1)
  4 stores (HWDGE/ACT): per ch: [(n cb):65536x128][c2l:2][2048 contig] 2 MB
"""

import numpy as np

import concourse.bacc as bacc
import concourse.bass as bass  # noqa: F401
import concourse.mybir as mybir
import concourse.tile as tile
from concourse.bass_utils import run_bass_kernel_spmd

N, C, H, W = 4, 256, 256, 256
P = 8          # patch size
GI = 32        # row blocks (i, scanned fastest in k)
GJ = 32        # col blocks (j)
IL = 4         # i-values per core (GI / 8 cores)
JQ = GJ // 4   # j per in-tile (quarter)

_NC = None


def _build():
    # Bacc (not plain Bass): its finalize() runs generate_event_semaphores,
    # which splits multi-wait instructions (HW allows 1 sync-wait per DMA).
    nc = bacc.Bacc()
    x = nc.dram_tensor(
        "x", [GJ, IL, N, 16384], mybir.dt.float32, kind="ExternalInput"
    )
    o = nc.dram_tensor("o", [N, C, GI, W], mybir.dt.float32, kind="ExternalOutput")
    # x5[i_loc, jq, q=(n cb), j, e]: per (i_loc, jq) a 3-dim AP
    #   [[512,128],[262144,8],[1,512]]
    x5 = x.rearrange("(jq j) i n (cb e) -> i jq (n cb) j e", jq=4, cb=GI)
    # o5[i_loc, ch, q=(n cb), c2l, rw]: per (i_loc, ch) a 3-dim AP
    #   [[65536,128],[8192,2],[1,2048]]
    o5 = o.rearrange("n (cb ch c2l) (i r) w -> i ch (n cb) c2l (r w)", ch=4, c2l=2, r=P)

    with tile.TileContext(nc) as tc:
        with (
            tc.tile_pool(name="tin", bufs=6) as pin,
            tc.tile_pool(name="tout", bufs=4) as pout,
        ):
            for i in range(IL):
                t_ins = []
                for jq in range(4):
                    t_in = pin.tile([128, JQ * 512], mybir.dt.float32)
                    nc.sync.dma_start(out=t_in[:], in_=x5[i, jq])
                    t_ins.append(t_in)
                # t_in laid out (j_loc, c2, a, b); view (q, c2, j_loc, a, b)
                tin_v = [
                    t[:].rearrange("q (j c2 a b) -> q c2 j a b", c2=P, a=P, b=P)
                    for t in t_ins
                ]
                for ch in range(4):
                    t_out = pout.tile([128, 2 * 2048], mybir.dt.float32)
                    # t_out laid out (c2l, a, j, b); view (q, c2l, jq, j, a, b)
                    tout_v = t_out[:].rearrange(
                        "q (c2l a jq j b) -> q c2l jq j a b",
                        c2l=2, a=P, jq=4, j=JQ, b=P,
                    )
                    for c2l in range(2):
                        for jq in range(4):
                            nc.vector.tensor_copy(
                                out=tout_v[:, c2l, jq],
                                in_=tin_v[jq][:, ch * 2 + c2l],
                            )
                    nc.scalar.dma_start(out=o5[i, ch], in_=t_out[:])
    nc.finalize()
    return nc


def _get_nc():
    global _NC
    if _NC is None:
        _NC = _build()
    return _NC


def _shard_inputs(x_np):
    v = x_np.reshape(N, GJ, GI, 16384)
    return [
        {
            "x": np.ascontiguousarray(
                v[:, :, ih * IL : (ih + 1) * IL, :].transpose(1, 2, 0, 3)
            )
        }
        for ih in range(8)
    ]


def _gather_outputs(results):
    out = np.empty((N, C, H, W), dtype=np.float32)
    for ih in range(8):
        out[:, :, ih * GI : (ih + 1) * GI, :] = results[ih]["o"]
    return out


def run(x_np, **spmd_kwargs):
    """Run on 8 cores; returns (out, BassKernelResults)."""
    nc = _get_nc()
    res = run_bass_kernel_spmd(
        nc, _shard_inputs(x_np), core_ids=list(range(8)), **spmd_kwargs
    )
    return _gather_outputs(res.results), res


def kernel(x, ori_shape=None, patch_size=None, **_):
    x_np = np.asarray(x, dtype=np.float32).reshape(N, 1024, 16384)
    out, _res = run(x_np)
    return out
